# revision 1
# baseline (speedup 1.0000x reference)
"""Trainium2 Bass kernel for AverageSpanExtractor (segment mean over spans).

Math note: the reference's masked softmax over all-ones logits reduces
exactly to a mean over the span tokens [start, end):
    out[b, n, :] = mean(sequence_tensor[b, start:end, :]).

Strategy (8 cores, batch-parallel - one batch element per core):
  1. Block-local prefix sums: per 128-token block, an inclusive
     triangular fp32 matmul (two blocks per instruction, [128,512]
     moving) produces rows R[128k+1 .. 128k+128] of a DRAM table `cum`
     stored in fp16. fp16 halves the store and gather-read traffic;
     max |R| < ~60 so fp16 keeps rel err ~1.3e-3, far inside 2e-2.
  2. Span sum = R[e] - R[s] + corr; a width<=32 span crosses at most
     one block boundary, corr = Ttab[(qe-qs)*qe] with q* = (x-1)>>7
     and Ttab[k] = cum[128k] (row 0 = zeros = no crossing).
  3. Phase 2 fetches R[e]/R[s] with four 512-row dma_gather ops on
     four parallel SWDGE queues (separate Q7 core pairs generate
     descriptors concurrently; random 512 B row reads are HBM
     transaction-bound, so queue depth matters). Indices are int16,
     host-scrambled into the 16-partition-wrapped layout the Q7 ucode
     consumes, replicated across all 128 partitions so every queue's
     core pair sees them.
  4. corr comes from the otherwise-idle PE: one-hot(gb) @ Ttab via a
     DVE compare + PE transpose + 33-row matmul per column, all
     overlapping the gathers. Combine = DVE subtract/add, 1/w scaling
     on the scalar engine, output stores on the idle SP ring.
"""

import numpy as np

B, S, D = 8, 4096, 256
N_SPANS = 1024
P = 128
NBLK = S // P          # 32 blocks of 128 tokens
JG = N_SPANS // P      # 8 spans per partition
MBK = 8                # token-blocks per load group
NT = 33                # block-total table rows (incl. zero row)

_cached_nc = None


def build_nc():
    import concourse.bass as bass
    import concourse.bacc as bacc
    import concourse.mybir as mybir
    from concourse.tile import TileContext
    from concourse.masks import make_upper_triangular, make_identity
    from concourse.instruction_name_ordered_set import InstructionNameOrderedSet

    def _nsdep(inst, *prereqs):
        s = InstructionNameOrderedSet()
        for p in prereqs:
            s.add(p.ins.name)
        inst.ins.add_nosync_dependencies_from(s)

    f32 = mybir.dt.float32
    f16 = mybir.dt.float16
    i32 = mybir.dt.int32
    i16 = mybir.dt.int16
    Alu = mybir.AluOpType
    Act = mybir.ActivationFunctionType

    nc = bacc.Bacc(
        None, target_bir_lowering=False, debug=False, num_devices=B,
        num_swdge_queues=4,
    )
    seq = nc.declare_dram_parameter("seq", [S, D], f32, isOutput=False)
    spans = nc.declare_dram_parameter("spans", [P, 2 * JG], i32, isOutput=False)
    # spans16[q, 32k + c*8 + r] = (k<2 ? end : start) of span
    # 8*(16r+q) + (k%2)*4 + c, replicated across the 8 16-row groups.
    spans16 = nc.declare_dram_parameter("spans16", [P, 128], i16, isOutput=False)
    out = nc.declare_dram_parameter("out", [N_SPANS, D], f32, isOutput=True)

    with TileContext(nc) as tc:
        with (
            tc.tile_pool(name="const", bufs=1) as const_pool,
            tc.tile_pool(name="x", bufs=4) as x_pool,
            tc.tile_pool(name="c", bufs=3) as c_pool,
            tc.tile_pool(name="ps", bufs=3, space="PSUM") as ps_pool,
            tc.tile_pool(name="oh", bufs=1, space="PSUM") as oh_pool,
            tc.tile_pool(name="cr", bufs=4, space="PSUM") as cr_pool,
            tc.tile_pool(name="misc", bufs=1) as misc_pool,
            tc.tile_pool(name="g", bufs=1) as g_pool,
            tc.tile_pool(name="res", bufs=1) as res_pool,
            tc.tile_pool(name="dram", bufs=1, space="DRAM") as d_pool,
        ):
            # DRAM scratch: block-local prefix rows in fp16; row 0 zeros.
            cum = d_pool.tile([S + 1, D], f16)

            tri = const_pool.tile([P, P], f32)
            make_upper_triangular(nc, tri[:], val=1.0, diag=True)
            ident = const_pool.tile([P, P], f16)
            make_identity(nc, ident[:])

            zrow = const_pool.tile([1, D], f16)
            nc.vector.memset(zrow[:], 0.0)
            nc.sync.dma_start(out=cum[0:1, :], in_=zrow[:])

            # Big seq loads FIRST on the sync queue: nothing needs the
            # span metadata early, and every 0.1 us here moves the whole
            # left edge of phase 1.
            bigxs = []
            for g in range(NBLK // MBK):
                t0 = g * MBK * P
                bigx = x_pool.tile([P, MBK * D], f32, name=f"bigx{g}")
                nc.sync.dma_start(
                    out=bigx[:],
                    in_=seq[t0 : t0 + MBK * P, :].rearrange(
                        "(m p) d -> p m d", p=P
                    ),
                )
                bigxs.append(bigx)

            # --- span index prep (overlaps phase 1) ---
            V = misc_pool.tile([P, 2 * JG], i32)
            nc.sync.dma_start(out=V[:], in_=spans[:])
            Sx = V[:, 0 : 2 * JG : 2]
            Ex = V[:, 1 : 2 * JG : 2]
            wi = misc_pool.tile([P, JG], i32)
            nc.vector.tensor_tensor(out=wi[:], in0=Ex, in1=Sx, op=Alu.subtract)
            wf = misc_pool.tile([P, JG], f32)
            nc.vector.tensor_copy(out=wf[:], in_=wi[:])
            wrec = misc_pool.tile([P, JG], f32)
            nc.vector.reciprocal(out=wrec[:], in_=wf[:])

            # gb = (qe - qs) * qe in 0..32; q* = (x-1)>>7 (arith shift).
            em1 = misc_pool.tile([P, JG], i32)
            nc.vector.tensor_scalar(
                out=em1[:], in0=Ex, scalar1=-1, scalar2=None, op0=Alu.add
            )
            qe = misc_pool.tile([P, JG], i32)
            nc.vector.tensor_scalar(
                out=qe[:], in0=em1[:], scalar1=7, scalar2=None,
                op0=Alu.arith_shift_right,
            )
            sm1 = misc_pool.tile([P, JG], i32)
            nc.vector.tensor_scalar(
                out=sm1[:], in0=Sx, scalar1=-1, scalar2=None, op0=Alu.add
            )
            qs = misc_pool.tile([P, JG], i32)
            nc.vector.tensor_scalar(
                out=qs[:], in0=sm1[:], scalar1=7, scalar2=None,
                op0=Alu.arith_shift_right,
            )
            dq = misc_pool.tile([P, JG], i32)
            nc.vector.tensor_tensor(out=dq[:], in0=qe[:], in1=qs[:], op=Alu.subtract)
            gb = misc_pool.tile([P, JG], i32)
            nc.vector.tensor_tensor(out=gb[:], in0=dq[:], in1=qe[:], op=Alu.mult)
            gbf = misc_pool.tile([P, JG], f16)
            nc.vector.tensor_copy(out=gbf[:], in_=gb[:])

            # gather indices: int16, host-scrambled; just load them.
            I16 = misc_pool.tile([P, 128], i16)
            nc.sync.dma_start(out=I16[:], in_=spans16[:])

            # Gather PREPS: descriptor generation is ~8.5 ns/row of strictly
            # serial Q7 time (~17 us for 2048 rows) -- run it NOW, under
            # phase 1, with prepare_only; the data read of `cum` defers to
            # the trigger_dma issued after the stores complete.
            G = g_pool.tile([P, 16 * D], f16)
            Gv = G[:].rearrange("p (c d) -> p c d", d=D)


            # --- phase 1: block-local prefix sums -> fp16 cum rows ---
            for g in range(NBLK // MBK):
                t0 = g * MBK * P
                bigx = bigxs[g]
                bigc = c_pool.tile([P, MBK * D], f16)
                for half in range(MBK // 2):
                    ps = ps_pool.tile([P, 2 * D], f32)
                    # two token-blocks per matmul: [128, 512] moving data
                    nc.tensor.matmul(
                        out=ps[:], lhsT=tri[:],
                        rhs=bigx[:, 2 * half * D : 2 * (half + 1) * D],
                        start=True, stop=True,
                    )
                    # fp32 PSUM -> fp16 SBUF; alternate DVE/ACT to balance
                    dst = bigc[:, 2 * half * D : 2 * (half + 1) * D]
                    if half % 2 == 0:
                        nc.vector.tensor_copy(out=dst, in_=ps[:])
                    else:
                        nc.scalar.activation(out=dst, in_=ps[:], func=Act.Copy)
                # one store per group on the SP ring: halves the sem
                # chain the first gather must wait through
                nc.sync.dma_start(
                    out=cum[1 + t0 : 1 + t0 + MBK * P, :].rearrange(
                        "(m p) d -> p m d", p=P
                    ),
                    in_=bigc[:],
                )

            # one-hot transposes for the correction matmuls (PE idle during the gathers)
            iota33 = misc_pool.tile([P, NT], i32)
            nc.gpsimd.iota(iota33[:], pattern=[[1, NT]], base=0, channel_multiplier=0)
            iota33f = misc_pool.tile([P, NT], f16)
            nc.vector.tensor_copy(out=iota33f[:], in_=iota33[:])
            ohS = []
            for j in range(JG):
                ohT = misc_pool.tile([P, NT], f16, name=f"ohT{j}")
                nc.vector.tensor_tensor(
                    out=ohT[:], in0=iota33f[:],
                    in1=gbf[:, j : j + 1].to_broadcast([P, NT]),
                    op=Alu.is_equal,
                )
                ohp = oh_pool.tile([NT, P], f16)
                nc.tensor.transpose(out=ohp[:], in_=ohT[:], identity=ident[:])
                ohSj = misc_pool.tile([NT, P], f16, name=f"ohS{j}")
                nc.vector.tensor_copy(out=ohSj[:], in_=ohp[:])
                ohS.append(ohSj)

            # block-total table: strided fetch of cum rows 0,128,...,4096
            Ttab = misc_pool.tile([NT, D], f16)
            nc.sync.dma_start(out=Ttab[:], in_=cum[0 : NBLK * P + 1 : P, :])

            # --- phase 2: fire the pre-generated gathers ---
            # Gate the trigger on all 9 cum-writing DMAs (zrow + 8 stores);
            # nosync dep pins the wait ahead of the trigger in the Pool
            # stream (the sequencer FIFO then enforces it).
            outv = out[:].rearrange("(p jj) d -> p jj d", p=P)
            # order E1,S1,E2,S2: the first combine half needs only the
            # first two gathers' rows, overlapping the remaining serial
            # descriptor generation.
            for q, k in enumerate((0, 2, 1, 3)):
                nc.gpsimd.dma_gather(
                    Gv[:, 4 * k : 4 * (k + 1), :],
                    cum[:],
                    I16[:, 32 * k : 32 * (k + 1)],
                    4 * P,
                    4 * P,
                    D,
                    queue_num=q,
                )
            # corr_j = onehot(gb_j) @ Ttab on the otherwise-idle PE.
            # Full-bank [P, 2D] tiles (half wasted): a PSUM bank written by
            # PE while DVE reads a bank-mate tile is a fatal HW collision,
            # so no two corr tiles may share a bank.
            corr = []
            for j in range(JG):
                crj = cr_pool.tile([P, 2 * D], f32)
                nc.tensor.matmul(
                    out=crj[:, 0:D], lhsT=ohS[j][:], rhs=Ttab[:],
                    start=True, stop=True,
                )
                corr.append(crj)

            T1 = res_pool.tile([P, 8 * D], f32)
            T1v = T1[:].rearrange("p (c d) -> p c d", d=D)
            R = res_pool.tile([P, 8 * D], f32)
            Rv = R[:].rearrange("p (c d) -> p c d", d=D)
            for h in range(2):
                # ends half h lives in Gv cols 4h..4h+4, starts in 8+4h..
                nc.vector.tensor_tensor(
                    out=T1v[:, 4 * h : 4 * h + 4, :],
                    in0=Gv[:, 4 * h : 4 * h + 4, :],
                    in1=Gv[:, 8 + 4 * h : 8 + 4 * h + 4, :],
                    op=Alu.subtract,
                )
                for c in range(4):
                    j = 4 * h + c
                    nc.vector.tensor_tensor(
                        out=T1v[:, j, :], in0=T1v[:, j, :], in1=corr[j][:, 0:D],
                        op=Alu.add,
                    )
                    nc.scalar.activation(
                        out=Rv[:, j, :], in_=T1v[:, j, :], func=Act.Copy,
                        scale=wrec[:, j : j + 1],
                    )
                nc.sync.dma_start(
                    out=outv[:, 4 * h : 4 * h + 4, :],
                    in_=Rv[:, 4 * h : 4 * h + 4, :],
                )
    nc.finalize()
    return nc


def _make_in_maps(sequence_tensor, span_indices):
    seq = np.ascontiguousarray(np.asarray(sequence_tensor), dtype=np.float32)
    si32 = np.asarray(span_indices).astype(np.int32)  # values <= 4096: lossless
    assert seq.shape == (B, S, D) and si32.shape == (B, N_SPANS, 2)
    in_maps = []
    for b in range(B):
        sv = si32[b].reshape(P, JG, 2)  # [p, j, (s, e)]
        # [q, c, r] scramble for the Q7 16-partition wrap; 4 blocks of 32
        # cols: [ends j0-3 | ends j4-7 | starts j0-3 | starts j4-7]
        g = sv.reshape(8, 16, JG, 2)  # [r, q, c, k]
        blocks = []
        for k in (1, 0):  # ends first, then starts
            for h in range(2):
                blk = g[:, :, 4 * h : 4 * h + 4, k]  # [r, q, 4]
                blocks.append(blk.transpose(1, 2, 0).reshape(16, 32))
        sp16 = np.concatenate(blocks, axis=1)  # [16, 128]
        sp16 = np.tile(sp16, (8, 1)).astype(np.int16)  # replicate to 128 rows
        in_maps.append(
            {
                "seq": seq[b],
                "spans": np.ascontiguousarray(si32[b].reshape(P, 2 * JG)),
                "spans16": np.ascontiguousarray(sp16),
            }
        )
    return in_maps


def kernel(sequence_tensor, span_indices):
    from concourse.bass_utils import run_bass_kernel_spmd

    global _cached_nc
    if _cached_nc is None:
        _cached_nc = build_nc()
    in_maps = _make_in_maps(sequence_tensor, span_indices)
    res = run_bass_kernel_spmd(_cached_nc, in_maps, list(range(B)))
    return np.stack([res.results[b]["out"] for b in range(B)], axis=0)

